# revision 8
# baseline (speedup 1.0000x reference)
"""IsoGMM loss kernel for 8 Trainium2 NeuronCores.

loss = mean_{n,k} r[n,k] * ||X[n] - mus[k]||^2

Decomposition (the entire loss folds into ONE accumulated PE matmul per core):
  sum_{n,k} r*d2 = T1 + T2 - 2*T3
    T1 = sum_n xsq_n * R_n        (xsq_n = ||X[n]||^2, R_n = sum_k r[n,k])
    T2 = sum_k musq_k * C_k       (C_k = sum_n r[n,k])
    T3 = sum_{k,d} mus[k,d] * M[k,d],  M = r.T @ X

Host augments X rows to width 130: [X | 1 | xsq-slot]; xsq is filled
on-chip (square the full contiguous row, then a per-row reduce: the
reduce then carries xsq + 1^2 + 0^2, and the stray +1 is cancelled
exactly by the musq-1 coefficient in the host-side combine). Per pair
of 128-row segments, one DoubleRow matmul:
  ps[64,130] += sum_i r_pair[:,i,:].T @ [X | 1 | xsq]_pair[:,i,:]
giving cols 0:128 = M, col 128 = C_k, col 129 = A_k = sum_n r[n,k]*xsq_n.

X and r are shipped in fp8 e4m3 (the 2e-2 rel-err budget dwarfs fp8
noise, simulated ~4.6e-3): quarters HBM traffic vs f32 and enables the
PE DoubleRow perf mode (256-row contraction per instruction). X rides
two DMA queues (sync/gpsimd issue alternate chunks), r a third
(scalar) - a single queue caps at ~226 GB/s.

Sharding: data-parallel over N, 16384 rows per core. Each SBUF partition
holds 128 *contiguous* rows (row order is irrelevant for every term), so
every DMA is perfectly contiguous per partition. Chunk sizes ramp up
(4,4,8,...) so compute starts right after the ~7us engine-init preamble.
"""

import numpy as np
import ml_dtypes

import concourse.bass as bass
import concourse.mybir as mybir
import concourse.tile as tile
from concourse import bacc
from concourse.bass_utils import run_bass_kernel_spmd

N, K, D = 131072, 64, 128
NCORES = 8
W = D + 2            # augmented row width: 128 data + ones + xsq slot
NS = N // NCORES     # rows per core
RPP = NS // 128      # rows per SBUF partition (= segments per core)
CHUNK_SIZES = (4, 4, 8, 8, 12, 12, 16, 16, 16, 16, 16)  # segments per chunk

FP8 = mybir.dt.float8e4
NP_FP8 = ml_dtypes.float8_e4m3


def build_nc(rpp=RPP, chunk_sizes=CHUNK_SIZES):
    segs = rpp
    assert sum(chunk_sizes) == segs
    xf = rpp * W
    rf = rpp * K
    f32 = mybir.dt.float32

    # Bacc (not plain Bass): its compile() splits sync waits to satisfy
    # TRN2's 1-wait-per-instruction limit, which walrus enforces.
    nc = bacc.Bacc("TRN2", target_bir_lowering=False, debug=False)
    xp = nc.dram_tensor("xp", [128, xf], FP8, kind="ExternalInput")
    rp = nc.dram_tensor("rp", [128, rf], FP8, kind="ExternalInput")
    out = nc.dram_tensor("out", [K, W], f32, kind="ExternalOutput")

    with (
        tile.TileContext(nc) as tc,
        tc.tile_pool(name="xb", bufs=5) as xpool,
        tc.tile_pool(name="rb", bufs=5) as rpool,
        tc.tile_pool(name="scr", bufs=4) as spool,
        tc.tile_pool(name="one", bufs=1) as onepool,
        tc.tile_pool(name="ps", bufs=1, space="PSUM") as pspool,
    ):
        ps = pspool.tile([K, W], f32)

        s0 = 0
        for c, spc in enumerate(chunk_sizes):
            xt = xpool.tile([128, spc * W], FP8, tag="x")
            rt = rpool.tile([128, spc * K], FP8, tag="r")
            xeng = nc.sync if c % 2 == 0 else nc.gpsimd
            xeng.dma_start(out=xt, in_=xp[:, s0 * W:(s0 + spc) * W])
            nc.scalar.dma_start(out=rt, in_=rp[:, s0 * K:(s0 + spc) * K])

            x3 = xt.rearrange("p (s w) -> p s w", w=W)
            r3 = rt.rearrange("p (s k) -> p s k", k=K)

            # per-row ||x||^2 + 1 via square of the full 130-wide rows
            # (contiguous stream - strided views halve throughput), then a
            # per-row reduce into the xsq slot. Square alternates
            # DVE/GPSIMD; the reduce is DVE-only capable.
            sq = spool.tile([128, spc * W], FP8, tag="sq")
            sq3 = sq.rearrange("p (s w) -> p s w", w=W)
            sq_eng = nc.vector if c % 2 == 0 else nc.gpsimd
            sq_eng.tensor_mul(sq, xt, xt)
            with nc.allow_low_precision(reason="xsq noise ~1e-4 of loss"):
                nc.vector.reduce_sum(
                    x3[:, :, D + 1:D + 2], sq3, axis=mybir.AxisListType.X
                )

            for j in range(0, spc, 2):
                s = s0 + j
                nc.tensor.matmul(
                    ps,
                    lhsT=r3[:, j:j + 2, :],
                    rhs=x3[:, j:j + 2, :],
                    start=(s == 0),
                    stop=(s == segs - 2),
                    perf_mode=mybir.MatmulPerfMode.DoubleRow,
                )
            s0 += spc

        # Ship the accumulated [K, W] panel; the final 64x130-element
        # weighted sum is part of host-side unsharding.
        osb = onepool.tile([K, W], f32)
        nc.vector.tensor_copy(osb, ps)
        nc.sync.dma_start(out=out[:, :], in_=osb)

    nc.compile()
    return nc


def make_in_maps(X, r, mus, ncores=NCORES):
    X = np.ascontiguousarray(np.asarray(X, dtype=np.float32))
    r = np.ascontiguousarray(np.asarray(r, dtype=np.float32))
    n = X.shape[0]
    ns = n // ncores

    Xb = X.astype(NP_FP8)
    rb = r.astype(NP_FP8)

    in_maps = []
    for i in range(ncores):
        Xa = np.empty((ns, W), NP_FP8)
        Xa[:, :D] = Xb[i * ns:(i + 1) * ns]
        Xa[:, D] = 1.0
        Xa[:, D + 1] = 0.0
        in_maps.append(
            {
                "xp": np.ascontiguousarray(Xa.reshape(128, (ns // 128) * W)),
                "rp": np.ascontiguousarray(
                    rb[i * ns:(i + 1) * ns].reshape(128, (ns // 128) * K)
                ),
            }
        )
    return in_maps


def combine_outputs(results, mus):
    """Unshard: weighted sum of each core's [K, W] panel -> mean."""
    mus = np.asarray(mus, dtype=np.float32)
    musq = (mus.astype(np.float64) ** 2).sum(1)
    # col 128 coefficient is musq-1: the on-chip xsq slot carries xsq+1
    # (the ones-column is part of the squared row), so A_k arrives as
    # A_k + C_k; the -1 on C_k cancels it exactly.
    ma = np.concatenate(
        [-2.0 * mus.astype(np.float64), musq[:, None] - 1.0, np.ones((K, 1))],
        axis=1,
    )
    total = 0.0
    for res in results:
        total += float((ma * res["out"].astype(np.float64)).sum())
    return np.array(total / (N * K), dtype=np.float32)


def kernel(X, r, mus):
    nc = build_nc()
    in_maps = make_in_maps(X, r, mus)
    res = run_bass_kernel_spmd(nc, in_maps, list(range(NCORES)))
    return combine_outputs(res.results[:NCORES], mus)


# revision 10
# speedup vs baseline: 1.3090x; 1.3090x over previous
"""IsoGMM loss kernel for 8 Trainium2 NeuronCores.

loss = mean_{n,k} r[n,k] * ||X[n] - mus[k]||^2

Decomposition (the loss folds into accumulated PE matmuls per core):
  sum_{n,k} r*d2 = T1 + T2 - 2*T3
    T1 = sum_n xsq_n * R_n        (xsq_n = ||X[n]||^2, R_n = sum_k r[n,k])
    T2 = sum_k musq_k * C_k       (C_k = sum_n r[n,k])
    T3 = sum_{k,d} mus[k,d] * M[k,d],  M = r.T @ X

Host augments X rows to width 130: [X | 1 | xsq-slot]. Per pair of
128-row segments, one DoubleRow matmul (256-row contraction):
  ps[64,130] += sum_i r_pair[:,i,:].T @ [X | 1 | xsq]_pair[:,i,:]
cols 0:128 = M, col 128 = C_k, col 129 = sum_n r[n,k]*(xsq_n+1).

The T1 ingredient sum_d X[n,d]^2 is the bottleneck (DVE's per-row
reduce runs at 1 elem/lane/cycle), so it is COMPUTED TWO WAYS to
balance engines:
 - path A (chunks in PATH_A): square full rows -> DVE reduce -> xsq
   slot, consumed by the main matmul (col 129).
 - path B (the rest): square full rows -> a SECOND accumulated matmul
   ps2[64,130] += r_pair.T @ sq_pair on the PE; sum_d ps2[k,d] =
   sum_n r[n,k]*(xsq_n + 1) for those rows (ones-col squared rides
   along). The xsq slot stays 0 so col 129 of ps is 0 for them.
Squares are split DVE/GPSIMD per SQ_DVE. The stray +1 per row (from
the squared ones-column) is cancelled exactly by the musq-1
coefficient in the host-side combine.

X and r are shipped in fp8 e4m3 (the 2e-2 rel-err budget dwarfs fp8
noise, simulated+measured ~4.6e-3): quarters HBM traffic vs f32 and
enables the PE DoubleRow perf mode. X rides the sync-issued DMA queue,
r the scalar-issued one.

Sharding: data-parallel over N, 16384 rows per core. Each SBUF
partition holds 128 *contiguous* rows (row order is irrelevant for
every term), so every DMA is perfectly contiguous per partition. Chunk
sizes ramp up (4,4,8,...) so compute starts right after the ~7us
engine-init preamble.
"""

import numpy as np
import ml_dtypes

import concourse.bass as bass
import concourse.mybir as mybir
import concourse.tile as tile
from concourse import bacc
from concourse.bass_utils import run_bass_kernel_spmd

N, K, D = 131072, 64, 128
NCORES = 8
W = D + 2            # augmented row width: 128 data + ones + xsq slot
NS = N // NCORES     # rows per core
RPP = NS // 128      # rows per SBUF partition (= segments per core)
# Sizes ramp up (fast pipeline fill after the ~7us preamble) and back
# down (short tail chain: the last chunk's square+matmuls gate the out
# DMA). Per-chunk knobs balance the engines: squares cost ~0.175us/seg
# on DVE and ~0.31us/seg on GPSIMD, the per-row reduce ~0.144us/seg on
# DVE, and each ps2 pair ~0.098us on the PE (which also carries the
# fixed 64 main pairs).
CHUNK_SIZES = (4, 4, 8, 8, 12, 16, 16, 16, 16, 12, 8, 4, 4)
PATH_A = frozenset({2})             # xsq via DVE reduce (8 segs)
SQ_DVE = frozenset({0, 2, 4, 6, 8, 9, 10, 11, 12})  # square on DVE (84
# segs); GPSIMD squares {1,3,5,7} (44 segs), kept off the tail chunks

FP8 = mybir.dt.float8e4
NP_FP8 = ml_dtypes.float8_e4m3


def build_nc(rpp=RPP, chunk_sizes=CHUNK_SIZES):
    segs = rpp
    assert sum(chunk_sizes) == segs
    xf = rpp * W
    rf = rpp * K
    f32 = mybir.dt.float32
    DR = mybir.MatmulPerfMode.DoubleRow

    # Bacc (not plain Bass): its compile() splits sync waits to satisfy
    # TRN2's 1-wait-per-instruction limit, which walrus enforces.
    nc = bacc.Bacc("TRN2", target_bir_lowering=False, debug=False)
    xp = nc.dram_tensor("xp", [128, xf], FP8, kind="ExternalInput")
    rp = nc.dram_tensor("rp", [128, rf], FP8, kind="ExternalInput")
    out = nc.dram_tensor("out", [K, 2 * W], f32, kind="ExternalOutput")

    n_b_pairs = sum(s // 2 for c, s in enumerate(chunk_sizes) if c not in PATH_A)

    with (
        tile.TileContext(nc) as tc,
        tc.tile_pool(name="xb", bufs=5) as xpool,
        tc.tile_pool(name="rb", bufs=5) as rpool,
        tc.tile_pool(name="scr", bufs=4) as spool,
        tc.tile_pool(name="one", bufs=1) as onepool,
        tc.tile_pool(name="ps", bufs=2, space="PSUM") as pspool,
    ):
        ps = pspool.tile([K, W], f32, tag="ps")
        ps2 = pspool.tile([K, W], f32, tag="ps2")

        s0 = 0
        b_pair = 0
        for c, spc in enumerate(chunk_sizes):
            xt = xpool.tile([128, spc * W], FP8, tag="x")
            rt = rpool.tile([128, spc * K], FP8, tag="r")
            nc.sync.dma_start(out=xt, in_=xp[:, s0 * W:(s0 + spc) * W])
            nc.scalar.dma_start(out=rt, in_=rp[:, s0 * K:(s0 + spc) * K])

            x3 = xt.rearrange("p (s w) -> p s w", w=W)
            r3 = rt.rearrange("p (s k) -> p s k", k=K)

            sq = spool.tile([128, spc * W], FP8, tag="sq")
            sq3 = sq.rearrange("p (s w) -> p s w", w=W)
            sq_eng = nc.vector if c in SQ_DVE else nc.gpsimd
            sq_eng.tensor_mul(sq, xt, xt)

            if c in PATH_A:
                with nc.allow_low_precision(reason="xsq noise ~1e-4 of loss"):
                    nc.vector.reduce_sum(
                        x3[:, :, D + 1:D + 2], sq3, axis=mybir.AxisListType.X
                    )

            for j in range(0, spc, 2):
                s = s0 + j
                nc.tensor.matmul(
                    ps,
                    lhsT=r3[:, j:j + 2, :],
                    rhs=x3[:, j:j + 2, :],
                    start=(s == 0),
                    stop=(s == segs - 2),
                    perf_mode=DR,
                )
                if c not in PATH_A:
                    nc.tensor.matmul(
                        ps2,
                        lhsT=r3[:, j:j + 2, :],
                        rhs=sq3[:, j:j + 2, :],
                        start=(b_pair == 0),
                        stop=(b_pair == n_b_pairs - 1),
                        perf_mode=DR,
                    )
                    b_pair += 1
            s0 += spc

        # Ship both accumulated [K, W] panels; the final weighted sum is
        # part of host-side unsharding.
        osb = onepool.tile([K, 2 * W], f32)
        nc.vector.tensor_copy(osb[:, 0:W], ps)
        nc.vector.tensor_copy(osb[:, W:2 * W], ps2)
        nc.sync.dma_start(out=out[:, :], in_=osb)

    nc.compile()
    return nc


def make_in_maps(X, r, mus, ncores=NCORES):
    X = np.ascontiguousarray(np.asarray(X, dtype=np.float32))
    r = np.ascontiguousarray(np.asarray(r, dtype=np.float32))
    n = X.shape[0]
    ns = n // ncores

    Xb = X.astype(NP_FP8)
    rb = r.astype(NP_FP8)

    in_maps = []
    for i in range(ncores):
        Xa = np.empty((ns, W), NP_FP8)
        Xa[:, :D] = Xb[i * ns:(i + 1) * ns]
        Xa[:, D] = 1.0
        Xa[:, D + 1] = 0.0
        in_maps.append(
            {
                "xp": np.ascontiguousarray(Xa.reshape(128, (ns // 128) * W)),
                "rp": np.ascontiguousarray(
                    rb[i * ns:(i + 1) * ns].reshape(128, (ns // 128) * K)
                ),
            }
        )
    return in_maps


def combine_outputs(results, mus):
    """Unshard: weighted sum of each core's panels -> mean."""
    mus = np.asarray(mus, dtype=np.float32)
    musq = (mus.astype(np.float64) ** 2).sum(1)
    # col 128 coefficient is musq-1: every row's T1 contribution carries
    # a stray +1 (the squared ones-column, via either the xsq slot or
    # the ps2 row-sum), cancelled exactly by the -1 on C_k here.
    ma = np.concatenate(
        [-2.0 * mus.astype(np.float64), musq[:, None] - 1.0, np.ones((K, 1))],
        axis=1,
    )
    total = 0.0
    for res in results:
        panel = res["out"].astype(np.float64)
        total += float((ma * panel[:, :W]).sum()) + float(panel[:, W:].sum())
    return np.array(total / (N * K), dtype=np.float32)


def kernel(X, r, mus):
    nc = build_nc()
    in_maps = make_in_maps(X, r, mus)
    res = run_bass_kernel_spmd(nc, in_maps, list(range(NCORES)))
    return combine_outputs(res.results[:NCORES], mus)


# revision 11
# speedup vs baseline: 1.5218x; 1.1626x over previous
"""IsoGMM loss kernel for 8 Trainium2 NeuronCores.

loss = mean_{n,k} r[n,k] * ||X[n] - mus[k]||^2

Decomposition (the loss folds into two accumulated PE matmuls per core):
  sum_{n,k} r*d2 = T1 + T2 - 2*T3
    T1 = sum_n xsq_n * R_n        (xsq_n = ||X[n]||^2, R_n = sum_k r[n,k])
    T2 = sum_k musq_k * C_k       (C_k = sum_n r[n,k])
    T3 = sum_{k,d} mus[k,d] * M[k,d],  M = r.T @ X

Host augments X rows to width 129: [X | 1]. Per pair of 128-row
segments, two DoubleRow matmuls (256-row contraction each):
  ps [64,129] += sum_i r_pair[:,i,:].T @ [X | 1]_pair[:,i,:]
  ps2[64,129] += sum_i r_pair[:,i,:].T @ ([X | 1]^2)_pair[:,i,:]
ps cols 0:128 = M, col 128 = C_k; sum_d ps2[k,d] = sum_n r[n,k]*
(xsq_n + 1) - the T1 ingredient, with the PE doing the per-row reduce
for free as part of the contraction. The stray +1 per row (squared
ones-column) is cancelled exactly by the musq-1 coefficient in the
host-side combine.

The elementwise square is the only remaining vector-engine pass; every
X element must cross a multiplier once and the PE can't square, so the
pass is split across DVE (~1.4ns/elem fp8), GPSIMD (~2.1ns/elem) and -
experimentally - the scalar/ACT engine per the SQ_* chunk maps.

X and r are shipped in fp8 e4m3 (the 2e-2 rel-err budget dwarfs fp8
noise, simulated+measured ~4e-3): quarters HBM traffic vs f32 and
enables the PE DoubleRow perf mode. X rides the sync-issued DMA queue,
r the scalar-issued one.

Sharding: data-parallel over N, 16384 rows per core. Each SBUF
partition holds 128 *contiguous* rows (row order is irrelevant for
every term), so every DMA is perfectly contiguous per partition. Chunk
sizes ramp up (fast pipeline fill after the ~7us engine-init preamble)
and back down (short tail chain: the last chunk's square+matmuls gate
the out DMA).
"""

import numpy as np
import ml_dtypes

import concourse.bass as bass
import concourse.mybir as mybir
import concourse.tile as tile
from concourse import bacc
from concourse.bass_utils import run_bass_kernel_spmd

N, K, D = 131072, 64, 128
NCORES = 8
W = D + 1            # augmented row width: 128 data + ones
NS = N // NCORES     # rows per core
RPP = NS // 128      # rows per SBUF partition (= segments per core)
CHUNK_SIZES = (4, 4, 8, 8, 12, 16, 16, 16, 16, 12, 8, 4, 4)
SQ_GPSIMD = frozenset({5, 7})       # 32 segs
SQ_ACT = frozenset({1, 3})          # 12 segs - ACT-square experiment
# remainder (84 segs) squares on DVE

FP8 = mybir.dt.float8e4
NP_FP8 = ml_dtypes.float8_e4m3


def build_nc(rpp=RPP, chunk_sizes=CHUNK_SIZES):
    segs = rpp
    assert sum(chunk_sizes) == segs
    xf = rpp * W
    rf = rpp * K
    f32 = mybir.dt.float32
    DR = mybir.MatmulPerfMode.DoubleRow

    # Bacc (not plain Bass): its compile() splits sync waits to satisfy
    # TRN2's 1-wait-per-instruction limit, which walrus enforces.
    nc = bacc.Bacc("TRN2", target_bir_lowering=False, debug=False)
    xp = nc.dram_tensor("xp", [128, xf], FP8, kind="ExternalInput")
    rp = nc.dram_tensor("rp", [128, rf], FP8, kind="ExternalInput")
    out = nc.dram_tensor("out", [K, 2 * W], f32, kind="ExternalOutput")

    with (
        tile.TileContext(nc) as tc,
        tc.tile_pool(name="xb", bufs=5) as xpool,
        tc.tile_pool(name="rb", bufs=5) as rpool,
        tc.tile_pool(name="scr", bufs=4) as spool,
        tc.tile_pool(name="one", bufs=1) as onepool,
        tc.tile_pool(name="ps", bufs=2, space="PSUM") as pspool,
    ):
        ps = pspool.tile([K, W], f32, tag="ps")
        ps2 = pspool.tile([K, W], f32, tag="ps2")

        s0 = 0
        for c, spc in enumerate(chunk_sizes):
            xt = xpool.tile([128, spc * W], FP8, tag="x")
            rt = rpool.tile([128, spc * K], FP8, tag="r")
            nc.sync.dma_start(out=xt, in_=xp[:, s0 * W:(s0 + spc) * W])
            nc.scalar.dma_start(out=rt, in_=rp[:, s0 * K:(s0 + spc) * K])

            x3 = xt.rearrange("p (s w) -> p s w", w=W)
            r3 = rt.rearrange("p (s k) -> p s k", k=K)

            sq = spool.tile([128, spc * W], FP8, tag="sq")
            sq3 = sq.rearrange("p (s w) -> p s w", w=W)
            if c in SQ_ACT:
                nc.scalar.square(sq, xt)
            else:
                sq_eng = nc.gpsimd if c in SQ_GPSIMD else nc.vector
                sq_eng.tensor_mul(sq, xt, xt)

            for j in range(0, spc, 2):
                s = s0 + j
                nc.tensor.matmul(
                    ps,
                    lhsT=r3[:, j:j + 2, :],
                    rhs=x3[:, j:j + 2, :],
                    start=(s == 0),
                    stop=(s == segs - 2),
                    perf_mode=DR,
                )
                nc.tensor.matmul(
                    ps2,
                    lhsT=r3[:, j:j + 2, :],
                    rhs=sq3[:, j:j + 2, :],
                    start=(s == 0),
                    stop=(s == segs - 2),
                    perf_mode=DR,
                )
            s0 += spc

        # Ship both accumulated [K, W] panels; the final weighted sum is
        # part of host-side unsharding.
        osb = onepool.tile([K, 2 * W], f32)
        nc.vector.tensor_copy(osb[:, 0:W], ps)
        nc.vector.tensor_copy(osb[:, W:2 * W], ps2)
        nc.sync.dma_start(out=out[:, :], in_=osb)

    nc.compile()
    return nc


def make_in_maps(X, r, mus, ncores=NCORES):
    X = np.ascontiguousarray(np.asarray(X, dtype=np.float32))
    r = np.ascontiguousarray(np.asarray(r, dtype=np.float32))
    n = X.shape[0]
    ns = n // ncores

    Xb = X.astype(NP_FP8)
    rb = r.astype(NP_FP8)

    in_maps = []
    for i in range(ncores):
        Xa = np.empty((ns, W), NP_FP8)
        Xa[:, :D] = Xb[i * ns:(i + 1) * ns]
        Xa[:, D] = 1.0
        in_maps.append(
            {
                "xp": np.ascontiguousarray(Xa.reshape(128, (ns // 128) * W)),
                "rp": np.ascontiguousarray(
                    rb[i * ns:(i + 1) * ns].reshape(128, (ns // 128) * K)
                ),
            }
        )
    return in_maps


def combine_outputs(results, mus):
    """Unshard: weighted sum of each core's panels -> mean."""
    mus = np.asarray(mus, dtype=np.float32)
    musq = (mus.astype(np.float64) ** 2).sum(1)
    # col 128 coefficient is musq-1: every row's T1 contribution (the
    # ps2 row-sum) carries a stray +1 from the squared ones-column,
    # cancelled exactly by the -1 on C_k here.
    ma = np.concatenate(
        [-2.0 * mus.astype(np.float64), musq[:, None] - 1.0], axis=1
    )
    total = 0.0
    for res in results:
        panel = res["out"].astype(np.float64)
        total += float((ma * panel[:, :W]).sum()) + float(panel[:, W:].sum())
    return np.array(total / (N * K), dtype=np.float32)


def kernel(X, r, mus):
    nc = build_nc()
    in_maps = make_in_maps(X, r, mus)
    res = run_bass_kernel_spmd(nc, in_maps, list(range(NCORES)))
    return combine_outputs(res.results[:NCORES], mus)


# revision 14
# speedup vs baseline: 1.5442x; 1.0147x over previous
"""IsoGMM loss kernel for 8 Trainium2 NeuronCores.

loss = mean_{n,k} r[n,k] * ||X[n] - mus[k]||^2

Decomposition (the loss folds into two accumulated PE matmuls per core):
  sum_{n,k} r*d2 = T1 + T2 - 2*T3
    T1 = sum_n xsq_n * R_n        (xsq_n = ||X[n]||^2, R_n = sum_k r[n,k])
    T2 = sum_k musq_k * C_k       (C_k = sum_n r[n,k])
    T3 = sum_{k,d} mus[k,d] * M[k,d],  M = r.T @ X

Host augments X rows to width 129: [X | 1]. Per pair of 128-row
segments, two DoubleRow matmuls (256-row contraction each):
  ps [64,129] += sum_i r_pair[:,i,:].T @ [X | 1]_pair[:,i,:]
  ps2[64,129] += sum_i r_pair[:,i,:].T @ ([X | 1]^2)_pair[:,i,:]
ps cols 0:128 = M, col 128 = C_k; sum_d ps2[k,d] = sum_n r[n,k]*
(xsq_n + 1) - the T1 ingredient, with the PE doing the per-row reduce
for free as part of the contraction. The stray +1 per row (squared
ones-column) is cancelled exactly by the musq-1 coefficient in the
host-side combine.

The elementwise square is the only remaining vector-engine pass; every
X element must cross a multiplier once and the PE can't square, so the
pass is split across DVE (~1.4ns/elem fp8), GPSIMD (~2.1ns/elem) and -
experimentally - the scalar/ACT engine per the SQ_* chunk maps.

X and r are shipped in fp8 e4m3 (the 2e-2 rel-err budget dwarfs fp8
noise, simulated+measured ~4e-3): quarters HBM traffic vs f32 and
enables the PE DoubleRow perf mode. X rides the sync-issued DMA queue,
r the scalar-issued one.

Sharding: data-parallel over N, 16384 rows per core. Each SBUF
partition holds 128 *contiguous* rows (row order is irrelevant for
every term), so every DMA is perfectly contiguous per partition. Chunk
sizes ramp up (fast pipeline fill after the ~7us engine-init preamble)
and back down (short tail chain: the last chunk's square+matmuls gate
the out DMA).
"""

import numpy as np
import ml_dtypes

import concourse.bass as bass
import concourse.mybir as mybir
import concourse.tile as tile
from concourse import bacc
from concourse.bass_utils import run_bass_kernel_spmd

N, K, D = 131072, 64, 128
NCORES = 8
W = D + 1            # augmented row width: 128 data + ones
NS = N // NCORES     # rows per core
RPP = NS // 128      # rows per SBUF partition (= segments per core)
CHUNK_SIZES = (4, 4, 8, 8, 12, 16, 16, 16, 16, 12, 8, 4, 4)
# Measured square rates: DVE ~0.18us/seg, ACT ~0.16, GPSIMD ~0.29.
# Three-way split puts every engine near ~8-10us, under the PE's ~13.
SQ_GPSIMD = frozenset({5, 9})            # 28 segs
SQ_ACT = frozenset({1, 3, 7, 8, 10, 11})  # 56 segs
# remainder (44 segs) squares on DVE
# r ships in 4 big DMAs (descriptor issue costs ~0.6us each on the
# issuing engine's sequencer; 13 of them was ~8us of scalar-engine
# time). Boundaries align with x-chunk edges. Issued by GPSIMD so the
# scalar engine keeps its cycles for ACT squares.
R_CHUNKS = ((0, 8), (8, 28), (36, 48), (84, 44))  # (start_seg, n_segs)

FP8 = mybir.dt.float8e4
NP_FP8 = ml_dtypes.float8_e4m3


def build_nc(rpp=RPP, chunk_sizes=CHUNK_SIZES):
    segs = rpp
    assert sum(chunk_sizes) == segs
    xf = rpp * W
    rf = rpp * K
    f32 = mybir.dt.float32
    DR = mybir.MatmulPerfMode.DoubleRow

    # Bacc (not plain Bass): its compile() splits sync waits to satisfy
    # TRN2's 1-wait-per-instruction limit, which walrus enforces.
    nc = bacc.Bacc("TRN2", target_bir_lowering=False, debug=False)
    xp = nc.dram_tensor("xp", [128, xf], FP8, kind="ExternalInput")
    rp = nc.dram_tensor("rp", [128, rf], FP8, kind="ExternalInput")
    out = nc.dram_tensor("out", [K, 2 * W], f32, kind="ExternalOutput")

    with (
        tile.TileContext(nc) as tc,
        tc.tile_pool(name="xb", bufs=5) as xpool,
        tc.tile_pool(name="rb", bufs=5) as rpool,
        tc.tile_pool(name="scr", bufs=4) as spool,
        tc.tile_pool(name="one", bufs=1) as onepool,
        tc.tile_pool(name="ps", bufs=2, space="PSUM") as pspool,
    ):
        ps = pspool.tile([K, W], f32, tag="ps")
        ps2 = pspool.tile([K, W], f32, tag="ps2")

        r_tiles = []
        for rs, rn in R_CHUNKS:
            rt = rpool.tile([128, rn * K], FP8, tag=f"r{rs}")
            nc.gpsimd.dma_start(out=rt, in_=rp[:, rs * K:(rs + rn) * K])
            r_tiles.append((rs, rn, rt.rearrange("p (s k) -> p s k", k=K)))

        def r_pair(s):
            for rs, rn, r3 in r_tiles:
                if rs <= s < rs + rn:
                    return r3[:, s - rs:s - rs + 2, :]
            raise AssertionError(s)

        s0 = 0
        for c, spc in enumerate(chunk_sizes):
            xt = xpool.tile([128, spc * W], FP8, tag="x")
            nc.sync.dma_start(out=xt, in_=xp[:, s0 * W:(s0 + spc) * W])

            x3 = xt.rearrange("p (s w) -> p s w", w=W)

            sq = spool.tile([128, spc * W], FP8, tag="sq")
            sq3 = sq.rearrange("p (s w) -> p s w", w=W)
            if c in SQ_ACT:
                nc.scalar.square(sq, xt)
            else:
                sq_eng = nc.gpsimd if c in SQ_GPSIMD else nc.vector
                sq_eng.tensor_mul(sq, xt, xt)

            for j in range(0, spc, 2):
                s = s0 + j
                lhsT = r_pair(s)
                nc.tensor.matmul(
                    ps,
                    lhsT=lhsT,
                    rhs=x3[:, j:j + 2, :],
                    start=(s == 0),
                    stop=(s == segs - 2),
                    perf_mode=DR,
                )
                nc.tensor.matmul(
                    ps2,
                    lhsT=lhsT,
                    rhs=sq3[:, j:j + 2, :],
                    start=(s == 0),
                    stop=(s == segs - 2),
                    perf_mode=DR,
                )
            s0 += spc

        # Ship both accumulated [K, W] panels; the final weighted sum is
        # part of host-side unsharding.
        osb = onepool.tile([K, 2 * W], f32)
        nc.vector.tensor_copy(osb[:, 0:W], ps)
        nc.vector.tensor_copy(osb[:, W:2 * W], ps2)
        nc.sync.dma_start(out=out[:, :], in_=osb)

    nc.compile()
    return nc


def make_in_maps(X, r, mus, ncores=NCORES):
    X = np.ascontiguousarray(np.asarray(X, dtype=np.float32))
    r = np.ascontiguousarray(np.asarray(r, dtype=np.float32))
    n = X.shape[0]
    ns = n // ncores

    Xb = X.astype(NP_FP8)
    rb = r.astype(NP_FP8)

    in_maps = []
    for i in range(ncores):
        Xa = np.empty((ns, W), NP_FP8)
        Xa[:, :D] = Xb[i * ns:(i + 1) * ns]
        Xa[:, D] = 1.0
        in_maps.append(
            {
                "xp": np.ascontiguousarray(Xa.reshape(128, (ns // 128) * W)),
                "rp": np.ascontiguousarray(
                    rb[i * ns:(i + 1) * ns].reshape(128, (ns // 128) * K)
                ),
            }
        )
    return in_maps


def combine_outputs(results, mus):
    """Unshard: weighted sum of each core's panels -> mean."""
    mus = np.asarray(mus, dtype=np.float32)
    musq = (mus.astype(np.float64) ** 2).sum(1)
    # col 128 coefficient is musq-1: every row's T1 contribution (the
    # ps2 row-sum) carries a stray +1 from the squared ones-column,
    # cancelled exactly by the -1 on C_k here.
    ma = np.concatenate(
        [-2.0 * mus.astype(np.float64), musq[:, None] - 1.0], axis=1
    )
    total = 0.0
    for res in results:
        panel = res["out"].astype(np.float64)
        total += float((ma * panel[:, :W]).sum()) + float(panel[:, W:].sum())
    return np.array(total / (N * K), dtype=np.float32)


def kernel(X, r, mus):
    nc = build_nc()
    in_maps = make_in_maps(X, r, mus)
    res = run_bass_kernel_spmd(nc, in_maps, list(range(NCORES)))
    return combine_outputs(res.results[:NCORES], mus)


# revision 15
# speedup vs baseline: 1.6577x; 1.0735x over previous
"""IsoGMM loss kernel for 8 Trainium2 NeuronCores.

loss = mean_{n,k} r[n,k] * ||X[n] - mus[k]||^2

Decomposition (the loss folds into two accumulated PE matmuls per core):
  sum_{n,k} r*d2 = T1 + T2 - 2*T3
    T1 = sum_n xsq_n * R_n        (xsq_n = ||X[n]||^2, R_n = sum_k r[n,k])
    T2 = sum_k musq_k * C_k       (C_k = sum_n r[n,k])
    T3 = sum_{k,d} mus[k,d] * M[k,d],  M = r.T @ X

Host augments X rows to width 129: [X | 1]. Per pair of 128-row
segments, two DoubleRow matmuls (256-row contraction each):
  ps [64,129] += sum_i r_pair[:,i,:].T @ [X | 1]_pair[:,i,:]
  ps2[64,129] += sum_i r_pair[:,i,:].T @ ([X | 1]^2)_pair[:,i,:]
ps cols 0:128 = M, col 128 = C_k; sum_d ps2[k,d] = sum_n r[n,k]*
(xsq_n + 1) - the T1 ingredient, with the PE doing the per-row reduce
for free as part of the contraction. The stray +1 per row (squared
ones-column) is cancelled exactly by the musq-1 coefficient in the
host-side combine.

The elementwise square is the only remaining vector-engine pass; every
X element must cross a multiplier once and the PE can't square, so the
pass is split across DVE (~1.4ns/elem fp8), GPSIMD (~2.1ns/elem) and -
experimentally - the scalar/ACT engine per the SQ_* chunk maps.

X and r are shipped in fp8 e4m3 (the 2e-2 rel-err budget dwarfs fp8
noise, simulated+measured ~4e-3): quarters HBM traffic vs f32 and
enables the PE DoubleRow perf mode. X rides the sync-issued DMA queue,
r the scalar-issued one.

Sharding: data-parallel over N, 16384 rows per core. Each SBUF
partition holds 128 *contiguous* rows (row order is irrelevant for
every term), so every DMA is perfectly contiguous per partition. Chunk
sizes ramp up (fast pipeline fill after the ~7us engine-init preamble)
and back down (short tail chain: the last chunk's square+matmuls gate
the out DMA).
"""

import numpy as np
import ml_dtypes

import concourse.bass as bass
import concourse.mybir as mybir
import concourse.tile as tile
from concourse import bacc
from concourse.bass_utils import run_bass_kernel_spmd

N, K, D = 131072, 64, 128
NCORES = 8
W = D + 1            # augmented row width: 128 data + ones
NS = N // NCORES     # rows per core
RPP = NS // 128      # rows per SBUF partition (= segments per core)
CHUNK_SIZES = (4, 4, 8, 8, 12, 16, 16, 16, 16, 12, 8, 4, 4)
# Measured square rates: DVE ~0.18us/seg, ACT ~0.16, GPSIMD ~0.29.
# Three-way split puts every engine near ~8-10us, under the PE's ~13.
SQ_GPSIMD = frozenset({5, 9})            # 28 segs
SQ_ACT = frozenset({1, 3, 7, 8, 10, 11})  # 56 segs
# remainder (44 segs) squares on DVE
# r ships in 4 big DMAs (descriptor issue costs ~0.6us each on the
# issuing engine's sequencer; 13 of them was ~8us of scalar-engine
# time). Boundaries align with x-chunk edges. Issued by GPSIMD so the
# scalar engine keeps its cycles for ACT squares.
R_CHUNKS = ((0, 8), (8, 28), (36, 48), (84, 44))  # (start_seg, n_segs)

FP8 = mybir.dt.float8e4
NP_FP8 = ml_dtypes.float8_e4m3


def build_nc(rpp=RPP, chunk_sizes=CHUNK_SIZES):
    segs = rpp
    assert sum(chunk_sizes) == segs
    xf = rpp * W
    rf = rpp * K
    f32 = mybir.dt.float32
    DR = mybir.MatmulPerfMode.DoubleRow

    # Bacc (not plain Bass): its compile() splits sync waits to satisfy
    # TRN2's 1-wait-per-instruction limit, which walrus enforces.
    nc = bacc.Bacc("TRN2", target_bir_lowering=False, debug=False)
    xp = nc.dram_tensor("xp", [128, xf], FP8, kind="ExternalInput")
    rp = nc.dram_tensor("rp", [128, rf], FP8, kind="ExternalInput")
    out = nc.dram_tensor("out", [K, 2 * W], f32, kind="ExternalOutput")

    with (
        tile.TileContext(nc) as tc,
        # Every chunk gets its own resident buffer (~41KB/partition total,
        # well under the 208KB budget): the DMA queue never stalls waiting
        # for compute to release a tile.
        tc.tile_pool(name="xb", bufs=len(CHUNK_SIZES)) as xpool,
        tc.tile_pool(name="rb", bufs=len(R_CHUNKS)) as rpool,
        tc.tile_pool(name="scr", bufs=len(CHUNK_SIZES)) as spool,
        tc.tile_pool(name="one", bufs=1) as onepool,
        tc.tile_pool(name="ps", bufs=2, space="PSUM") as pspool,
    ):
        ps = pspool.tile([K, W], f32, tag="ps")
        ps2 = pspool.tile([K, W], f32, tag="ps2")

        r_tiles = []
        for rs, rn in R_CHUNKS:
            rt = rpool.tile([128, rn * K], FP8, tag=f"r{rs}")
            nc.gpsimd.dma_start(out=rt, in_=rp[:, rs * K:(rs + rn) * K])
            r_tiles.append((rs, rn, rt.rearrange("p (s k) -> p s k", k=K)))

        def r_pair(s):
            for rs, rn, r3 in r_tiles:
                if rs <= s < rs + rn:
                    return r3[:, s - rs:s - rs + 2, :]
            raise AssertionError(s)

        s0 = 0
        for c, spc in enumerate(chunk_sizes):
            xt = xpool.tile([128, spc * W], FP8, tag="x")
            nc.sync.dma_start(out=xt, in_=xp[:, s0 * W:(s0 + spc) * W])

            x3 = xt.rearrange("p (s w) -> p s w", w=W)

            sq = spool.tile([128, spc * W], FP8, tag="sq")
            sq3 = sq.rearrange("p (s w) -> p s w", w=W)
            if c in SQ_ACT:
                nc.scalar.square(sq, xt)
            else:
                sq_eng = nc.gpsimd if c in SQ_GPSIMD else nc.vector
                sq_eng.tensor_mul(sq, xt, xt)

            for j in range(0, spc, 2):
                s = s0 + j
                lhsT = r_pair(s)
                nc.tensor.matmul(
                    ps,
                    lhsT=lhsT,
                    rhs=x3[:, j:j + 2, :],
                    start=(s == 0),
                    stop=(s == segs - 2),
                    perf_mode=DR,
                )
                nc.tensor.matmul(
                    ps2,
                    lhsT=lhsT,
                    rhs=sq3[:, j:j + 2, :],
                    start=(s == 0),
                    stop=(s == segs - 2),
                    perf_mode=DR,
                )
            s0 += spc

        # Ship both accumulated [K, W] panels; the final weighted sum is
        # part of host-side unsharding.
        osb = onepool.tile([K, 2 * W], f32)
        nc.vector.tensor_copy(osb[:, 0:W], ps)
        nc.vector.tensor_copy(osb[:, W:2 * W], ps2)
        nc.sync.dma_start(out=out[:, :], in_=osb)

    nc.compile()
    return nc


def make_in_maps(X, r, mus, ncores=NCORES):
    X = np.ascontiguousarray(np.asarray(X, dtype=np.float32))
    r = np.ascontiguousarray(np.asarray(r, dtype=np.float32))
    n = X.shape[0]
    ns = n // ncores

    Xb = X.astype(NP_FP8)
    rb = r.astype(NP_FP8)

    in_maps = []
    for i in range(ncores):
        Xa = np.empty((ns, W), NP_FP8)
        Xa[:, :D] = Xb[i * ns:(i + 1) * ns]
        Xa[:, D] = 1.0
        in_maps.append(
            {
                "xp": np.ascontiguousarray(Xa.reshape(128, (ns // 128) * W)),
                "rp": np.ascontiguousarray(
                    rb[i * ns:(i + 1) * ns].reshape(128, (ns // 128) * K)
                ),
            }
        )
    return in_maps


def combine_outputs(results, mus):
    """Unshard: weighted sum of each core's panels -> mean."""
    mus = np.asarray(mus, dtype=np.float32)
    musq = (mus.astype(np.float64) ** 2).sum(1)
    # col 128 coefficient is musq-1: every row's T1 contribution (the
    # ps2 row-sum) carries a stray +1 from the squared ones-column,
    # cancelled exactly by the -1 on C_k here.
    ma = np.concatenate(
        [-2.0 * mus.astype(np.float64), musq[:, None] - 1.0], axis=1
    )
    total = 0.0
    for res in results:
        panel = res["out"].astype(np.float64)
        total += float((ma * panel[:, :W]).sum()) + float(panel[:, W:].sum())
    return np.array(total / (N * K), dtype=np.float32)


def kernel(X, r, mus):
    nc = build_nc()
    in_maps = make_in_maps(X, r, mus)
    res = run_bass_kernel_spmd(nc, in_maps, list(range(NCORES)))
    return combine_outputs(res.results[:NCORES], mus)
